# revision 1
# baseline (speedup 1.0000x reference)
"""BitNet FFN Trainium2 kernel (8-core SPMD, data-parallel over tokens).

Math (forward values of the STE reference):
  wq(w)  = clip(round(w/s), -1, 1) * s,  s = mean(|w|) + EPS        (ternary)
  xq(x)  = round(x/sx) * sx,  sx = max(absmax_row(x), EPS)/127      (int8 range)
  gate = sigmoid(xq @ wq_g.T); up = xq @ wq_u.T; h = gate*up
  out  = hq(h) @ wq_d.T

Strategy: every matmul runs in bf16 with fp32 PSUM accumulation on exact
integers (|int| <= 127 activations, ternary weights, partial sums < 2^24),
so the integer matmuls are exact; all scales are folded in fp32 outside the
matmuls. Tokens are sharded 8 ways (1024/core); each core streams the full
weights once. The only collective is a 16-byte AllReduce for the three
global weight-scale sums.
"""

import sys

sys.path.insert(0, "/opt/trn_rl_repo")

import numpy as np

import concourse.tile as tile
from concourse import bacc, mybir

F32 = mybir.dt.float32
BF16 = mybir.dt.bfloat16
ADD = mybir.AluOpType.add
SUB = mybir.AluOpType.subtract
MULT = mybir.AluOpType.mult
MAX = mybir.AluOpType.max
AXX = mybir.AxisListType.X
AFT = mybir.ActivationFunctionType

EPS = 1e-5
CR = 12582912.0  # 1.5*2^23: fp32 RNE round-to-integer magic constant
ALPHA = 1.0986122886681098  # atanh(0.5)/0.5 : tanh(ALPHA*0.5) == 0.5
P = 128


def build_program(T, DM, FF, ncores, ff_sh, dm_sh):
    """Build the per-core SPMD program.

    T: tokens per core; DM: d_model; FF: d_ff; ff_sh/dm_sh: rows of the
    per-core weight-scale shards (w_gate/w_up shard rows, w_down shard rows).
    """
    assert T % P == 0 and DM % P == 0 and FF % 1024 == 0
    MT = T // P              # token tiles
    KD = DM // P             # d_model k-blocks
    NG = FF // 1024          # phase-1 ff groups (8 strips each)
    K3 = FF // P             # phase-3 ff k-blocks
    MD = DM // P             # output dm blocks
    TN = min(512, T)         # moving free dim (tokens) per matmul
    NT3 = T // TN            # phase-3 token chunks
    WPC = min(2048, DM)      # scale-pass piece width for g/u
    WPC3 = min(2048, FF)     # scale-pass piece width for wd

    nc = bacc.Bacc(
        "TRN2",
        target_bir_lowering=False,
        debug=False,
        enable_asserts=False,
        num_devices=ncores,
    )

    x_d = nc.dram_tensor("x", [T, DM], F32, kind="ExternalInput")
    wg_d = nc.dram_tensor("wg", [FF, DM], F32, kind="ExternalInput")
    wu_d = nc.dram_tensor("wu", [FF, DM], F32, kind="ExternalInput")
    wd_d = nc.dram_tensor("wd", [DM, FF], F32, kind="ExternalInput")
    wgs_d = nc.dram_tensor("wg_sh", [ff_sh, DM], F32, kind="ExternalInput")
    wus_d = nc.dram_tensor("wu_sh", [ff_sh, DM], F32, kind="ExternalInput")
    wds_d = nc.dram_tensor("wd_sh", [dm_sh, FF], F32, kind="ExternalInput")
    out_d = nc.dram_tensor("out_t", [DM, T], F32, kind="ExternalOutput")

    NW = float(FF * DM)  # elements per weight matrix (all three equal)

    with tile.TileContext(nc, num_cores=ncores) as tc:
        import contextlib

        with contextlib.ExitStack() as outer:
            dram = outer.enter_context(tc.tile_pool(name="dram", bufs=1, space="DRAM"))
            psum = outer.enter_context(tc.tile_pool(name="psum", bufs=8, space="PSUM"))
            tiny = outer.enter_context(tc.tile_pool(name="tiny", bufs=1))

            hp_d = dram.tile([T, FF], F32)       # h' = sigmoid(G)*U_int
            shs_d = dram.tile([1, T], F32)       # per-token output scale row
            cc_in = dram.tile([1, 4], F32)
            cc_out = dram.tile([1, 4], F32)

            # persistent small tiles
            ones_col = tiny.tile([P, 1], F32)
            nc.vector.memset(ones_col, 1.0)
            ones_row = tiny.tile([1, P], F32)
            nc.vector.memset(ones_row, 1.0)
            sb_scales = tiny.tile([P, 8], F32)   # bcast: bg,bu,bd,swg,swu,swd
            sx_all = tiny.tile([P, MT], F32)     # per-token x scale (col=token tile)
            rx_all = tiny.tile([P, MT], F32)
            sxg_all = tiny.tile([P, MT], F32)    # sx*swg (sigmoid input scale)
            sxu_all = tiny.tile([P, MT], F32)    # sx*swu
            rph_all = tiny.tile([P, MT], F32)    # s_xu/s_h (h' quant scale)
            shd_all = tiny.tile([P, MT], F32)    # s_h*s_wd (output scale)
            accs = tiny.tile([P, MT, 2 * NG], F32)  # h' absmax partials

            # ---------------- S0: global weight scales ----------------
            with tc.tile_pool(name="s0", bufs=3) as s0p, tc.tile_pool(
                name="s0t", bufs=4
            ) as s0t:
                acc3 = tiny.tile([P, 4], F32)
                nc.vector.memset(acc3, 0.0)
                shard_specs = [
                    (wgs_d, 0, ff_sh, DM, WPC),
                    (wus_d, 1, ff_sh, DM, WPC),
                    (wds_d, 2, dm_sh, FF, WPC3),
                ]
                for src, col, rows, cols, pw in shard_specs:
                    for r0 in range(0, rows, P):
                        pr = min(P, rows - r0)
                        for c0 in range(0, cols, pw):
                            t_in = s0p.tile([P, pw], F32, name="s0raw")
                            nc.sync.dma_start(
                                t_in[:pr], src[r0 : r0 + pr, c0 : c0 + pw]
                            )
                            t_abs = s0p.tile([P, pw], F32, name="s0abs")
                            t_sum = s0t.tile([P, 1], F32, name="s0sum")
                            nc.scalar.activation(
                                out=t_abs[:pr],
                                in_=t_in[:pr],
                                func=AFT.Abs,
                                accum_out=t_sum[:pr],
                            )
                            nc.vector.tensor_tensor(
                                out=acc3[:pr, col : col + 1],
                                in0=acc3[:pr, col : col + 1],
                                in1=t_sum[:pr],
                                op=ADD,
                            )
                ps_s = psum.tile([P, 512], F32, name="ps_main")
                nc.tensor.matmul(
                    ps_s[:4, :1], acc3[:, :4], ones_col, start=True, stop=True
                )
                sb_s = s0t.tile([4, 1], F32, name="sb_s")
                nc.vector.tensor_copy(sb_s, ps_s[:4, :1])
                nc.sync.dma_start(cc_in[0, :4], sb_s[:, 0])
                nc.gpsimd.collective_compute(
                    "AllReduce",
                    ADD,
                    replica_groups=[list(range(ncores))],
                    ins=[cc_in[:].opt()],
                    outs=[cc_out[:].opt()],
                )
                sums_row = s0t.tile([1, 4], F32, name="sums_row")
                nc.sync.dma_start(sums_row, cc_out[:])
                sw_row = s0t.tile([1, 4], F32, name="sw_row")
                nc.vector.tensor_scalar(
                    out=sw_row, in0=sums_row, scalar1=1.0 / NW, scalar2=EPS,
                    op0=MULT, op1=ADD,
                )
                beta_row = s0t.tile([1, 4], F32, name="beta_row")
                nc.vector.reciprocal(beta_row, sw_row)
                row8 = s0t.tile([1, 8], F32, name="row8")
                nc.vector.tensor_scalar(
                    out=row8[:, 0:4], in0=beta_row, scalar1=ALPHA, scalar2=None,
                    op0=MULT, op1=mybir.AluOpType.bypass,
                )
                nc.vector.tensor_copy(row8[:, 4:8], sw_row)
                ps_b = psum.tile([P, 512], F32, name="ps_main")
                nc.tensor.matmul(
                    ps_b[:, :8], ones_row, row8, start=True, stop=True
                )
                nc.vector.tensor_copy(sb_scales, ps_b[:, :8])

            # ---------------- phase 0/1: x-quant + gate/up + h' ----------------
            with contextlib.ExitStack() as ph1:
                xqt_p = ph1.enter_context(tc.tile_pool(name="xqt", bufs=1))

                xqt = xqt_p.tile([P, KD, T], BF16)  # XqT: [dm-part, k, token]

                # x quantization (per token-tile) in its own pool scope
                with tc.tile_pool(name="xw", bufs=3) as xw_p:
                    for m in range(MT):
                        xt = xw_p.tile([P, DM], F32, name="xt")
                        nc.gpsimd.dma_start(xt, x_d[m * P : (m + 1) * P, :])
                        amax = xw_p.tile([P, 1], F32, name="amax")
                        nc.vector.tensor_reduce(
                            amax, xt, axis=AXX, op=MAX, apply_absolute_value=True
                        )
                        nc.vector.tensor_scalar(
                            out=sx_all[:, m : m + 1], in0=amax, scalar1=EPS,
                            scalar2=1.0 / 127.0, op0=MAX, op1=MULT,
                        )
                        nc.vector.reciprocal(
                            rx_all[:, m : m + 1], sx_all[:, m : m + 1]
                        )
                        nc.vector.tensor_tensor(
                            out=sxg_all[:, m : m + 1], in0=sx_all[:, m : m + 1],
                            in1=sb_scales[:, 4:5], op=MULT,
                        )
                        nc.vector.tensor_tensor(
                            out=sxu_all[:, m : m + 1], in0=sx_all[:, m : m + 1],
                            in1=sb_scales[:, 5:6], op=MULT,
                        )
                        xr = xw_p.tile([P, DM], F32, name="xr")
                        nc.vector.tensor_scalar(
                            out=xr, in0=xt, scalar1=rx_all[:, m : m + 1], scalar2=CR,
                            op0=MULT, op1=ADD,
                        )
                        xq = xw_p.tile([P, DM], BF16, name="xq")
                        nc.vector.tensor_scalar(
                            out=xq, in0=xr, scalar1=CR, scalar2=None,
                            op0=SUB, op1=mybir.AluOpType.bypass,
                        )
                        nc.sync.dma_start_transpose(
                            xqt[:, :, m * P : (m + 1) * P], xq
                        )

                wraw_p = ph1.enter_context(tc.tile_pool(name="wraw", bufs=3))
                wtern_p = ph1.enter_context(tc.tile_pool(name="wtern", bufs=3))
                wchunk_p = ph1.enter_context(tc.tile_pool(name="wchunk", bufs=6))
                gate_p = ph1.enter_context(
                    tc.tile_pool(name="gate", bufs=4)
                )
                hpr_p = ph1.enter_context(tc.tile_pool(name="hpr", bufs=2))
                sc_p = ph1.enter_context(tc.tile_pool(name="scp", bufs=2))

                # Merged gate+up pass per 512-ff group (4 strips each).
                # Ternary chunks are strip-major [P, strip(4), k(KD), 128] so
                # each strip transpose lands contiguous. One LDWEIGHTS (xqT
                # tile) feeds the G and U matmuls; 2 PSUM banks per token
                # tile so four token tiles pipeline.
                def produce_chunk(eng, wsrc, beta_col, ng):
                    chunk = wchunk_p.tile([P, 4, KD, P], BF16, name="wchunk")
                    for s4 in range(4):
                        r0 = (ng * 4 + s4) * P
                        raw = wraw_p.tile([P, DM], F32, name="wraw")
                        nc.gpsimd.dma_start(raw, wsrc[r0 : r0 + P, :])
                        nc.scalar.activation(
                            out=raw, in_=raw, func=AFT.Tanh,
                            scale=sb_scales[:, beta_col : beta_col + 1],
                        )
                        tern = wtern_p.tile([P, DM], BF16, name="wtern")
                        nc.vector.tensor_scalar(
                            out=tern, in0=raw, scalar1=CR, scalar2=CR,
                            op0=ADD, op1=SUB,
                        )
                        eng.dma_start_transpose(
                            chunk[:, s4 : s4 + 1, :, :], tern
                        )
                    return chunk

                NG5 = FF // 512
                for ng in range(NG5):
                    chunk_g = produce_chunk(nc.sync, wg_d, 0, ng)
                    chunk_u = produce_chunk(nc.sync, wu_d, 1, ng)
                    for m in range(MT):
                        psg = psum.tile([P, 512], F32, name="ps_main")
                        psu = psum.tile([P, 512], F32, name="ps_main")
                        for k in range(KD):
                            lhsT = xqt[:, k, m * P : (m + 1) * P]
                            st, sp = (k == 0), (k == KD - 1)
                            nc.tensor.matmul(
                                psg, lhsT, chunk_g[:, :, k, :], start=st, stop=sp
                            )
                            nc.tensor.matmul(
                                psu, lhsT, chunk_u[:, :, k, :], start=st, stop=sp
                            )
                        gt = gate_p.tile([P, 512], F32, name="gate_t")
                        nc.scalar.activation(
                            out=gt, in_=psg, func=AFT.Sigmoid,
                            scale=sxg_all[:, m : m + 1],
                        )
                        hp = hpr_p.tile([P, 512], F32, name="hp")
                        nc.vector.tensor_tensor(out=hp, in0=gt, in1=psu, op=MULT)
                        nc.vector.tensor_reduce(
                            accs[:, m, ng : ng + 1], hp, axis=AXX,
                            op=MAX, apply_absolute_value=True,
                        )
                        nc.scalar.dma_start(
                            hp_d[m * P : (m + 1) * P, ng * 512 : (ng + 1) * 512],
                            hp,
                        )

                # h scales per token tile
                for m in range(MT):
                    am = sc_p.tile([P, 1], F32, name="am")
                    nc.vector.tensor_reduce(
                        am, accs[:, m, :], axis=AXX, op=MAX
                    )
                    nc.vector.tensor_tensor(
                        out=am, in0=am, in1=sxu_all[:, m : m + 1], op=MULT
                    )
                    sh = sc_p.tile([P, 1], F32, name="sh")
                    nc.vector.tensor_scalar(
                        out=sh, in0=am, scalar1=EPS, scalar2=1.0 / 127.0,
                        op0=MAX, op1=MULT,
                    )
                    rs = sc_p.tile([P, 1], F32, name="rs")
                    nc.vector.reciprocal(rs, sh)
                    nc.vector.tensor_tensor(
                        out=rph_all[:, m : m + 1], in0=rs,
                        in1=sxu_all[:, m : m + 1], op=MULT,
                    )
                    nc.vector.tensor_tensor(
                        out=shd_all[:, m : m + 1], in0=sh,
                        in1=sb_scales[:, 6:7], op=MULT,
                    )
                    nc.sync.dma_start(
                        shs_d[0, m * P : (m + 1) * P], shd_all[:, m : m + 1]
                    )

            # ---------------- phase 2/3: quantize h' + down projection ----------------
            with contextlib.ExitStack() as ph23:
                hqtb_p = ph23.enter_context(tc.tile_pool(name="hqtb", bufs=1))
                # hqt: [ff-in-block, ff-block k, token] — transposed quantized h
                hqt = hqtb_p.tile([P, K3, T], BF16)

                # S5: quantize h' into hqt, ff-column-major so phase-3 matmuls
                # can consume early k columns while later ones still quantize
                with tc.tile_pool(name="s5", bufs=6) as s5p:
                    PW5 = min(2048, FF)
                    for c0 in range(0, FF, PW5):
                        for m in range(MT):
                            hpt = s5p.tile([P, PW5], F32, name="hpt")
                            nc.gpsimd.dma_start(
                                hpt, hp_d[m * P : (m + 1) * P, c0 : c0 + PW5]
                            )
                            nc.vector.tensor_scalar(
                                out=hpt, in0=hpt, scalar1=rph_all[:, m : m + 1],
                                scalar2=CR, op0=MULT, op1=ADD,
                            )
                            hqq = s5p.tile([P, PW5], BF16, name="hqq")
                            nc.vector.tensor_scalar(
                                out=hqq, in0=hpt, scalar1=CR, scalar2=None,
                                op0=SUB, op1=mybir.AluOpType.bypass,
                            )
                            nc.sync.dma_start_transpose(
                                hqt[
                                    :,
                                    c0 // P : (c0 + PW5) // P,
                                    m * P : (m + 1) * P,
                                ],
                                hqq,
                            )

                shs_p = ph23.enter_context(tc.tile_pool(name="shsp", bufs=1))
                wdr_p = ph23.enter_context(tc.tile_pool(name="wdr", bufs=2))
                wdtern_p = ph23.enter_context(tc.tile_pool(name="wdtn", bufs=1))
                wdt_p = ph23.enter_context(tc.tile_pool(name="wdtg", bufs=3))
                fin_p = ph23.enter_context(tc.tile_pool(name="finp", bufs=2))

                shs_row = shs_p.tile([1, T], F32, name="shs_row")
                nc.sync.dma_start(shs_row, shs_d[:])
                shs_bc = shs_p.tile([P, T], F32, name="shs_bc")
                for t in range(NT3):
                    ps_bc = psum.tile([P, 512], F32, name="ps_main")
                    nc.tensor.matmul(
                        ps_bc[:, :TN], ones_row,
                        shs_row[:, t * TN : (t + 1) * TN], start=True, stop=True,
                    )
                    nc.vector.tensor_copy(
                        shs_bc[:, t * TN : (t + 1) * TN], ps_bc[:, :TN]
                    )

                # fused: ternarize+transpose w_down per output dm-block,
                # full-k PSUM accumulation; emitted inside the s5 scope so
                # wd production and early matmuls overlap quantization
                KH = K3 // 2  # k-blocks per wdtg half-tile
                for md in range(MD):
                    halves = []
                    for h in range(2):
                        wdtg = wdt_p.tile([P, KH, P], BF16, name="wdtg")
                        halves.append(wdtg)
                        base = h * (FF // 2)
                        PW3 = min(2048, FF // 2)
                        for c0 in range(0, FF // 2, PW3):
                            raw = wdr_p.tile([P, PW3], F32, name="wdraw")
                            nc.gpsimd.dma_start(
                                raw,
                                wd_d[
                                    md * P : (md + 1) * P,
                                    base + c0 : base + c0 + PW3,
                                ],
                            )
                            nc.scalar.activation(
                                out=raw, in_=raw, func=AFT.Tanh,
                                scale=sb_scales[:, 2:3],
                            )
                            ternd = wdtern_p.tile([P, PW3], BF16, name="wdtern")
                            nc.vector.tensor_scalar(
                                out=ternd, in0=raw, scalar1=CR, scalar2=CR,
                                op0=ADD, op1=SUB,
                            )
                            nc.sync.dma_start_transpose(
                                wdtg[:, c0 // P : (c0 + PW3) // P, :], ternd
                            )
                    pss = [
                        psum.tile([P, 512], F32, name="ps_main")
                        for _ in range(NT3)
                    ]
                    for k in range(K3):
                        lhsT = halves[k // KH][:, k % KH, :]
                        for t in range(NT3):
                            nc.tensor.matmul(
                                pss[t][:, :TN],
                                lhsT,
                                hqt[:, k, t * TN : (t + 1) * TN],
                                start=(k == 0),
                                stop=(k == K3 - 1),
                            )
                    for t in range(NT3):
                        ot = fin_p.tile([P, TN], F32, name="ot")
                        nc.vector.tensor_tensor(
                            out=ot, in0=pss[t][:, :TN],
                            in1=shs_bc[:, t * TN : (t + 1) * TN], op=MULT,
                        )
                        nc.scalar.dma_start(
                            out_d[md * P : (md + 1) * P, t * TN : (t + 1) * TN],
                            ot,
                        )

    nc.compile()
    return nc


_CACHE = {}
TRACE = False  # set True (e.g. from test.py) to capture an NTFF profile
LAST_RESULTS = None


def _get_program(T, DM, FF, ncores, ff_sh, dm_sh):
    key = (T, DM, FF, ncores, ff_sh, dm_sh)
    if key not in _CACHE:
        _CACHE[key] = build_program(T, DM, FF, ncores, ff_sh, dm_sh)
    return _CACHE[key]


def kernel(x, w_gate, w_up, w_down):
    from concourse.bass_utils import run_bass_kernel_spmd

    x = np.asarray(x, dtype=np.float32)
    w_gate = np.ascontiguousarray(np.asarray(w_gate, dtype=np.float32))
    w_up = np.ascontiguousarray(np.asarray(w_up, dtype=np.float32))
    w_down = np.ascontiguousarray(np.asarray(w_down, dtype=np.float32))

    B, S, DM = x.shape
    FF = w_gate.shape[0]
    NCORES = 8
    NTOK = B * S
    T = NTOK // NCORES
    ff_sh = FF // NCORES
    dm_sh = DM // NCORES

    xf = np.ascontiguousarray(x.reshape(NTOK, DM))
    nc = _get_program(T, DM, FF, NCORES, ff_sh, dm_sh)

    in_maps = []
    for c in range(NCORES):
        in_maps.append(
            {
                "x": np.ascontiguousarray(xf[c * T : (c + 1) * T]),
                "wg": w_gate,
                "wu": w_up,
                "wd": w_down,
                "wg_sh": np.ascontiguousarray(w_gate[c * ff_sh : (c + 1) * ff_sh]),
                "wu_sh": np.ascontiguousarray(w_up[c * ff_sh : (c + 1) * ff_sh]),
                "wd_sh": np.ascontiguousarray(w_down[c * dm_sh : (c + 1) * dm_sh]),
            }
        )

    res = run_bass_kernel_spmd(
        nc, in_maps, core_ids=list(range(NCORES)), trace=TRACE
    )
    global LAST_RESULTS
    LAST_RESULTS = res
    out = np.empty((NTOK, DM), dtype=np.float32)
    for c in range(NCORES):
        out[c * T : (c + 1) * T] = res.results[c]["out_t"].T
    return out.reshape(B, S, DM)



# revision 34
# speedup vs baseline: 1.4274x; 1.4274x over previous
"""BitNet FFN Trainium2 kernel (8-core SPMD, data-parallel over tokens).

Math (forward values of the STE reference):
  wq(w)  = clip(round(w/s), -1, 1) * s,  s = mean(|w|) + EPS        (ternary)
  xq(x)  = round(x/sx) * sx,  sx = max(absmax_row(x), EPS)/127      (int8 range)
  gate = sigmoid(xq @ wq_g.T); up = xq @ wq_u.T; h = gate*up
  out  = hq(h) @ wq_d.T

Strategy: every matmul runs in bf16 with fp32 PSUM accumulation on exact
integers (|int| <= 127 activations, ternary weights, partial sums < 2^24),
so the integer matmuls are exact; all scales are folded in fp32 outside the
matmuls. Tokens are sharded 8 ways (1024/core); each core streams the full
weights once.

Transpose-free dataflow: the host passes pre-transposed layouts (x^T,
w_gate^T, w_up^T, w_down^T), the ternarize pass is orientation-agnostic,
and phase 1 computes G^T/U^T = w_tern.T @ xq^T with the ternary weight tile
stationary -- so h'^T lands directly in the [ff, token] layout that the
down-projection contracts over. No dma_start_transpose anywhere; the only
cross-partition step is one gpsimd partition_all_reduce for the per-token
h absmax. The only collective is a 16-byte AllReduce for the three global
weight-scale sums.
"""

import sys

sys.path.insert(0, "/opt/trn_rl_repo")

import contextlib

import numpy as np

import concourse.tile as tile
from concourse import bacc, bass_isa, mybir

F32 = mybir.dt.float32
BF16 = mybir.dt.bfloat16
ADD = mybir.AluOpType.add
SUB = mybir.AluOpType.subtract
MULT = mybir.AluOpType.mult
MAX = mybir.AluOpType.max
MIN = mybir.AluOpType.min
BYP = mybir.AluOpType.bypass
AXX = mybir.AxisListType.X
AFT = mybir.ActivationFunctionType

EPS = 1e-5
CR = 12582912.0  # 1.5*2^23: fp32 RNE round-to-integer magic constant
ALPHA = 1.0986122886681098  # 2*atanh(0.5): tanh(ALPHA*0.5) == 0.5
P = 128


def build_program(T, DM, FF, ncores, ff_sh, dm_sh):
    """Build the per-core SPMD program.

    T: tokens per core; DM: d_model; FF: d_ff; ff_sh/dm_sh: rows of the
    per-core weight-scale shards (w_gate/w_up shard rows, w_down shard rows).
    """
    KD = DM // P             # d_model k-blocks
    TN = min(512, T)         # moving free dim (tokens) per matmul
    NTC = T // TN            # token chunks
    RW = 512                 # phase-1 ff range width
    NR = FF // RW            # phase-1 ranges
    FBR = RW // P            # ff blocks per range
    K3 = FF // P             # phase-3 ff k-blocks
    MD = DM // P             # output dm blocks
    MDW = min(4, MD)         # dm blocks per phase-3 group
    NMDG = MD // MDW
    RW3 = MDW * P            # phase-3 wd slab width (dm)
    assert T % P == 0 and DM % P == 0 and FF % RW == 0 and MD % MDW == 0

    nc = bacc.Bacc(
        "TRN2",
        target_bir_lowering=False,
        debug=False,
        enable_asserts=False,
        num_devices=ncores,
    )

    x_d = nc.dram_tensor("x", [T, DM], F32, kind="ExternalInput")
    xt_d = nc.dram_tensor("xt", [DM, T], F32, kind="ExternalInput")
    wgt_d = nc.dram_tensor("wgt", [DM, FF], F32, kind="ExternalInput")
    wut_d = nc.dram_tensor("wut", [DM, FF], F32, kind="ExternalInput")
    wdt_d = nc.dram_tensor("wdt", [FF, DM], F32, kind="ExternalInput")
    wgs_d = nc.dram_tensor("wg_sh", [ff_sh, DM], F32, kind="ExternalInput")
    wus_d = nc.dram_tensor("wu_sh", [ff_sh, DM], F32, kind="ExternalInput")
    wds_d = nc.dram_tensor("wd_sh", [dm_sh, FF], F32, kind="ExternalInput")
    out_d = nc.dram_tensor("out_t", [DM, T], F32, kind="ExternalOutput")

    NW = float(FF * DM)  # elements per weight matrix (all three equal)

    with tile.TileContext(nc, num_cores=ncores) as tc:
        with contextlib.ExitStack() as outer:
            dram = outer.enter_context(
                tc.tile_pool(name="dram", bufs=1, space="DRAM")
            )
            psum = outer.enter_context(
                tc.tile_pool(
                    name="psum", bufs=(8 if TN <= 512 else 4), space="PSUM"
                )
            )
            tiny = outer.enter_context(tc.tile_pool(name="tiny", bufs=1))

            hp_d = dram.tile([FF, T], F32)       # h'^T = (sigmoid(G)*U)^T
            wdq_d = dram.tile([FF, DM], BF16)    # pre-ternarized w_down^T
            cc_in = dram.tile([1, 4], F32)
            cc_out = dram.tile([1, 4], F32)
            ccw_in = dram.tile([1, 4], F32)
            ccw_out = dram.tile([1, 4], F32)

            # warm-up collective: absorbs the CC path's first-op setup
            # latency in parallel with the weight-scale scan below
            nc.gpsimd.collective_compute(
                "AllReduce",
                ADD,
                replica_groups=[list(range(ncores))],
                ins=[ccw_in[:].opt()],
                outs=[ccw_out[:].opt()],
            )

            ones_col = tiny.tile([P, 1], F32)
            nc.vector.memset(ones_col, 1.0)
            cr_col = tiny.tile([P, 1], F32)
            nc.vector.memset(cr_col, CR)
            ncr_col = tiny.tile([P, 1], F32)
            nc.vector.memset(ncr_col, -CR)
            ones_row = tiny.tile([1, P], F32)
            nc.vector.memset(ones_row, 1.0)
            sb_scales = tiny.tile([P, 8], F32)   # 0-2: beta*ALPHA g/u/d; 4-6: sw
            # h' running max/min (abs_max alu op doesn't lower in codegen)
            MP = tiny.tile([P, T], F32)          # max h', [ff-part, tok]
            nc.vector.memset(MP, -1e30)
            MN = tiny.tile([P, T], F32)          # min h'
            nc.vector.memset(MN, 1e30)

            # ---------------- phases 0/1: x-quant + gate/up -> h'^T ---------
            with contextlib.ExitStack() as ph1:
                bcp = ph1.enter_context(tc.tile_pool(name="bcp", bufs=3))
                sx_bc = bcp.tile([P, T], F32, name="bc")
                rx_bc = bcp.tile([P, T], F32, name="bc")
                sxg_bc = bcp.tile([P, T], F32, name="bc")
                # sxu_bc persists: the phase-3 boundary folds s_xu into the
                # h quant scale (h is stored at integer scale)
                sxu_bc = tiny.tile([P, T], F32)

                xq_p = ph1.enter_context(tc.tile_pool(name="xqp", bufs=KD))
                xqt = [xq_p.tile([P, T], BF16, name="xqt") for _ in range(KD)]

                # x quantization: absmax from natural layout, quantize the
                # host-transposed copy with a per-token broadcast scale.
                with tc.tile_pool(name="xnat", bufs=2) as xn_p, tc.tile_pool(
                    name="xts", bufs=3
                ) as xt_p, tc.tile_pool(name="sxc", bufs=1) as sxc_p:
                    MT = T // P
                    sx_cols = sxc_p.tile([P, MT], F32)
                    sx_row = sxc_p.tile([1, T], F32)
                    for m in range(MT):
                        xnat = xn_p.tile([P, DM], F32, name="xnat")
                        nc.scalar.dma_start(xnat, x_d[m * P : (m + 1) * P, :])
                        amax = sxc_p.tile([P, 1], F32, name="amax")
                        nc.vector.tensor_reduce(
                            amax, xnat, axis=AXX, op=MAX,
                            apply_absolute_value=True,
                        )
                        nc.vector.tensor_scalar(
                            out=sx_cols[:, m : m + 1], in0=amax, scalar1=EPS,
                            scalar2=1.0 / 127.0, op0=MAX, op1=MULT,
                        )
                        nc.scalar.dma_start(
                            sx_row[0:1, m * P : (m + 1) * P],
                            sx_cols[:, m : m + 1],
                        )
                    # broadcast sx_row across partitions via the (idle) PE;
                    # gpsimd's FIFO is blocked behind the collective here
                    for t in range(T // TN):
                        ps_bc = psum.tile([P, TN], F32, name="ps_main")
                        nc.tensor.matmul(
                            ps_bc, ones_row,
                            sx_row[0:1, t * TN : (t + 1) * TN],
                            start=True, stop=True,
                        )
                        nc.vector.tensor_copy(
                            sx_bc[:, t * TN : (t + 1) * TN], ps_bc
                        )
                    nc.vector.reciprocal(rx_bc, sx_bc)
                    for k in range(KD):
                        xts = xt_p.tile([P, T], F32, name="xts")
                        nc.scalar.dma_start(xts, xt_d[k * P : (k + 1) * P, :])
                        nc.vector.tensor_tensor(
                            out=xts, in0=xts, in1=rx_bc, op=MULT
                        )
                        nc.vector.tensor_scalar(
                            out=xqt[k], in0=xts, scalar1=CR, scalar2=CR,
                            op0=ADD, op1=SUB,
                        )

            # ---------------- S0: global weight scales ----------------
                with tc.tile_pool(name="s0", bufs=3) as s0p, tc.tile_pool(
                    name="s0t", bufs=8
                ) as s0t:
                    acc3 = tiny.tile([P, 4], F32)
                    nc.vector.memset(acc3, 0.0)
                    shard_specs = [
                        (wgs_d, 0, ff_sh, DM),
                        (wus_d, 1, ff_sh, DM),
                        (wds_d, 2, dm_sh, FF),
                    ]
                    idx = 0
                    for src, col, rows, cols in shard_specs:
                        pw = min(2048, cols)
                        for r0 in range(0, rows, P):
                            pr = min(P, rows - r0)
                            for c0 in range(0, cols, pw):
                                t_in = s0p.tile([P, pw], F32, name="s0raw")
                                nc.sync.dma_start(
                                    t_in[:pr], src[r0 : r0 + pr, c0 : c0 + pw]
                                )
                                t_sum = s0t.tile([P, 1], F32, name="s0sum")
                                # scalar engine only: keeps the DVE free for
                                # x-quant during the collective window
                                t_abs = s0p.tile([P, pw], F32, name="s0abs")
                                nc.scalar.activation(
                                    out=t_abs[:pr],
                                    in_=t_in[:pr],
                                    func=AFT.Abs,
                                    accum_out=t_sum[:pr],
                                )
                                idx += 0  # (engine split removed)
                                nc.vector.tensor_tensor(
                                    out=acc3[:pr, col : col + 1],
                                    in0=acc3[:pr, col : col + 1],
                                    in1=t_sum[:pr],
                                    op=ADD,
                                )
                                idx += 1
                    ps_s = psum.tile([P, TN], F32, name="ps_main")
                    nc.tensor.matmul(
                        ps_s[:4, :1], acc3[:, :4], ones_col, start=True, stop=True
                    )
                    sb_s = s0t.tile([4, 1], F32, name="sb_s")
                    nc.vector.tensor_copy(sb_s, ps_s[:4, :1])
                    nc.sync.dma_start(cc_in[0, :4], sb_s[:, 0])
                    nc.gpsimd.collective_compute(
                        "AllReduce",
                        ADD,
                        replica_groups=[list(range(ncores))],
                        ins=[cc_in[:].opt()],
                        outs=[cc_out[:].opt()],
                    )
                    sums_row = s0t.tile([1, 4], F32, name="sums_row")
                    nc.sync.dma_start(sums_row, cc_out[:])
                    sw_row = s0t.tile([1, 4], F32, name="sw_row")
                    nc.vector.tensor_scalar(
                        out=sw_row, in0=sums_row, scalar1=1.0 / NW, scalar2=EPS,
                        op0=MULT, op1=ADD,
                    )
                    beta_row = s0t.tile([1, 4], F32, name="beta_row")
                    nc.vector.reciprocal(beta_row, sw_row)
                    row8 = s0t.tile([1, 8], F32, name="row8")
                    nc.vector.tensor_scalar(
                        out=row8[:, 0:4], in0=beta_row, scalar1=ALPHA, scalar2=None,
                        op0=MULT, op1=BYP,
                    )
                    nc.vector.tensor_copy(row8[:, 4:8], sw_row)
                    ps_b = psum.tile([P, TN], F32, name="ps_main")
                    nc.tensor.matmul(
                        ps_b[:, :8], ones_row, row8, start=True, stop=True
                    )
                    nc.vector.tensor_copy(sb_scales, ps_b[:, :8])

                nc.vector.tensor_scalar(
                    out=sxg_bc, in0=sx_bc, scalar1=sb_scales[:, 4:5],
                    scalar2=None, op0=MULT, op1=BYP,
                )
                nc.vector.tensor_scalar(
                    out=sxu_bc, in0=sx_bc, scalar1=sb_scales[:, 5:6],
                    scalar2=None, op0=MULT, op1=BYP,
                )

                tern_p = ph1.enter_context(tc.tile_pool(name="tern", bufs=48))
                wraw_p = ph1.enter_context(tc.tile_pool(name="wraw", bufs=8))
                hst_p = ph1.enter_context(tc.tile_pool(name="hst", bufs=6))
                wdp_r = ph1.enter_context(tc.tile_pool(name="wdpr", bufs=2))
                wdp_t = ph1.enter_context(tc.tile_pool(name="wdpt", bufs=2))

                K3R = K3 // NR  # w_down k-strips pre-ternarized per range
                for r in range(NR):
                    ternG = [
                        tern_p.tile([P, RW], BF16, name="tern")
                        for _ in range(KD)
                    ]
                    ternU = [
                        tern_p.tile([P, RW], BF16, name="tern")
                        for _ in range(KD)
                    ]
                    for k in range(KD):
                        rawg = wraw_p.tile([P, RW], F32, name="wraw")
                        nc.gpsimd.dma_start(
                            rawg,
                            wgt_d[k * P : (k + 1) * P, r * RW : (r + 1) * RW],
                        )
                        nc.scalar.activation(
                            out=rawg, in_=rawg, func=AFT.Tanh,
                            scale=sb_scales[:, 0:1],
                        )
                        nc.vector.tensor_scalar(
                            out=ternG[k], in0=rawg, scalar1=CR, scalar2=CR,
                            op0=ADD, op1=SUB,
                        )
                        rawu = wraw_p.tile([P, RW], F32, name="wraw")
                        nc.gpsimd.dma_start(
                            rawu,
                            wut_d[k * P : (k + 1) * P, r * RW : (r + 1) * RW],
                        )
                        nc.scalar.activation(
                            out=rawu, in_=rawu, func=AFT.Tanh,
                            scale=sb_scales[:, 1:2],
                        )
                        nc.vector.tensor_scalar(
                            out=ternU[k], in0=rawu, scalar1=CR, scalar2=CR,
                            op0=ADD, op1=SUB,
                        )
                    for fb in range(FBR):
                        psG = [
                            psum.tile([P, TN], F32, name="ps_main")
                            for _ in range(NTC)
                        ]
                        psU = [
                            psum.tile([P, TN], F32, name="ps_main")
                            for _ in range(NTC)
                        ]
                        for k in range(KD):
                            lhsG = ternG[k][:, fb * P : (fb + 1) * P]
                            for t in range(NTC):
                                nc.tensor.matmul(
                                    psG[t], lhsG,
                                    xqt[k][:, t * TN : (t + 1) * TN],
                                    start=(k == 0), stop=(k == KD - 1),
                                )
                        for k in range(KD):
                            lhsU = ternU[k][:, fb * P : (fb + 1) * P]
                            for t in range(NTC):
                                nc.tensor.matmul(
                                    psU[t], lhsU,
                                    xqt[k][:, t * TN : (t + 1) * TN],
                                    start=(k == 0), stop=(k == KD - 1),
                                )
                        # h is kept at integer scale (gate * U_int); the
                        # missing s_xu factor folds into the phase-3 scales.
                        fr = (r * FBR + fb) * P
                        for t in range(NTC):
                            ts0, ts1 = t * TN, (t + 1) * TN
                            gi = hst_p.tile([P, TN], F32, name="hst")
                            nc.vector.tensor_tensor(
                                out=gi, in0=psG[t], in1=sxg_bc[:, ts0:ts1],
                                op=MULT,
                            )
                            nc.scalar.activation(
                                out=gi, in_=gi, func=AFT.Sigmoid
                            )
                            uu = hst_p.tile([P, TN], F32, name="hst")
                            nc.vector.tensor_tensor(
                                out=uu, in0=gi, in1=psU[t], op=MULT
                            )
                            nc.vector.tensor_tensor(
                                out=MP[:, ts0:ts1], in0=MP[:, ts0:ts1],
                                in1=uu, op=MAX,
                            )
                            nc.vector.tensor_tensor(
                                out=MN[:, ts0:ts1], in0=MN[:, ts0:ts1],
                                in1=uu, op=MIN,
                            )
                            nc.scalar.dma_start(
                                hp_d[fr : fr + P, ts0:ts1], uu
                            )
                    # pre-ternarize this range's strips of w_down into DRAM
                    # (uses phase-1 engine slack; phase 3 loads them ready)
                    for j in range(K3R):
                        k3 = r * K3R + j
                        wdraw = wdp_r.tile([P, DM], F32, name="wdraw")
                        nc.gpsimd.dma_start(
                            wdraw, wdt_d[k3 * P : (k3 + 1) * P, :]
                        )
                        nc.scalar.activation(
                            out=wdraw, in_=wdraw, func=AFT.Tanh,
                            scale=sb_scales[:, 2:3],
                        )
                        wdtn = wdp_t.tile([P, DM], BF16, name="wdtn")
                        nc.vector.tensor_scalar(
                            out=wdtn, in0=wdraw, scalar1=CR, scalar2=CR,
                            op0=ADD, op1=SUB,
                        )
                        nc.scalar.dma_start(
                            wdq_d[k3 * P : (k3 + 1) * P, :], wdtn
                        )

            # ---------------- phase 2/3: h-quant + down projection ----------
            with contextlib.ExitStack() as ph3:
                # per-token scales, already broadcast across partitions:
                # absmax_int = max(|max|, |min|) reduced over partitions,
                # then true absmax = absmax_int * s_xu (h was stored at
                # integer scale).
                MPr = tiny.tile([P, T], F32)
                nc.gpsimd.partition_all_reduce(
                    MPr, MP, channels=P, reduce_op=bass_isa.ReduceOp.absmax
                )
                MNr = tiny.tile([P, T], F32)
                nc.gpsimd.partition_all_reduce(
                    MNr, MN, channels=P, reduce_op=bass_isa.ReduceOp.absmax
                )
                nc.vector.tensor_tensor(out=MPr, in0=MPr, in1=MNr, op=MAX)
                nc.vector.tensor_tensor(out=MPr, in0=MPr, in1=sxu_bc, op=MULT)
                sh_bc = MNr  # reuse the buffer for s_h
                nc.vector.tensor_scalar(
                    out=sh_bc, in0=MPr, scalar1=EPS, scalar2=1.0 / 127.0,
                    op0=MAX, op1=MULT,
                )
                ch_bc = tiny.tile([P, T], F32)  # h_int quant scale s_xu/s_h
                nc.vector.reciprocal(ch_bc, sh_bc)
                nc.vector.tensor_tensor(
                    out=ch_bc, in0=ch_bc, in1=sxu_bc, op=MULT
                )
                shd_bc = tiny.tile([P, T], F32)  # output scale s_h * s_wd
                nc.vector.tensor_scalar(
                    out=shd_bc, in0=sh_bc, scalar1=sb_scales[:, 6:7],
                    scalar2=None, op0=MULT, op1=BYP,
                )

                hq_p = ph3.enter_context(tc.tile_pool(name="hqp", bufs=K3))
                hps_p = ph3.enter_context(tc.tile_pool(name="hps", bufs=4))
                wdr_p = ph3.enter_context(tc.tile_pool(name="wdr", bufs=6))
                fin_p = ph3.enter_context(tc.tile_pool(name="fin", bufs=4))

                hqt = [hq_p.tile([P, T], BF16, name="hqt") for _ in range(K3)]
                for mdg in range(NMDG):
                    pss = [
                        psum.tile([P, TN], F32, name="ps_main")
                        for _ in range(MDW * NTC)
                    ]
                    for k3 in range(K3):
                        if mdg == 0:
                            # quantize h just-in-time, interleaved with the
                            # first group's matmuls so the PE never drains;
                            # the CR round is split DVE/scalar to balance
                            # engine load (DVE also carries the scale mult)
                            hst = hps_p.tile([P, T], F32, name="hp3")
                            nc.sync.dma_start(
                                hst, hp_d[k3 * P : (k3 + 1) * P, :]
                            )
                            nc.vector.tensor_tensor(
                                out=hst, in0=hst, in1=ch_bc, op=MULT
                            )
                            HF = max(32, (3 * T // 8) // 32 * 32)
                            nc.vector.tensor_scalar(
                                out=hqt[k3][:, :HF], in0=hst[:, :HF],
                                scalar1=CR, scalar2=CR, op0=ADD, op1=SUB,
                            )
                            nc.scalar.activation(
                                out=hst[:, HF:], in_=hst[:, HF:],
                                func=AFT.Identity, bias=cr_col[:, 0:1],
                            )
                            nc.scalar.activation(
                                out=hqt[k3][:, HF:], in_=hst[:, HF:],
                                func=AFT.Identity, bias=ncr_col[:, 0:1],
                            )
                        ternD = wdr_p.tile([P, RW3], BF16, name="ternD")
                        nc.gpsimd.dma_start(
                            ternD,
                            wdq_d[
                                k3 * P : (k3 + 1) * P,
                                mdg * RW3 : (mdg + 1) * RW3,
                            ],
                        )
                        for md in range(MDW):
                            lhsD = ternD[:, md * P : (md + 1) * P]
                            for t in range(NTC):
                                nc.tensor.matmul(
                                    pss[md * NTC + t], lhsD,
                                    hqt[k3][:, t * TN : (t + 1) * TN],
                                    start=(k3 == 0), stop=(k3 == K3 - 1),
                                )
                    for md in range(MDW):
                        dr = (mdg * MDW + md) * P
                        for t in range(NTC):
                            o = fin_p.tile([P, TN], F32, name="ot")
                            nc.vector.tensor_tensor(
                                out=o, in0=pss[md * NTC + t],
                                in1=shd_bc[:, t * TN : (t + 1) * TN], op=MULT,
                            )
                            nc.scalar.dma_start(
                                out_d[dr : dr + P, t * TN : (t + 1) * TN], o
                            )

    nc.compile()
    return nc


_CACHE = {}
TRACE = False  # set True (e.g. from test.py) to capture an NTFF profile
LAST_RESULTS = None


def _get_program(T, DM, FF, ncores, ff_sh, dm_sh):
    key = (T, DM, FF, ncores, ff_sh, dm_sh)
    if key not in _CACHE:
        _CACHE[key] = build_program(T, DM, FF, ncores, ff_sh, dm_sh)
    return _CACHE[key]


def kernel(x, w_gate, w_up, w_down):
    from concourse.bass_utils import run_bass_kernel_spmd

    x = np.asarray(x, dtype=np.float32)
    w_gate = np.ascontiguousarray(np.asarray(w_gate, dtype=np.float32))
    w_up = np.ascontiguousarray(np.asarray(w_up, dtype=np.float32))
    w_down = np.ascontiguousarray(np.asarray(w_down, dtype=np.float32))

    B, S, DM = x.shape
    FF = w_gate.shape[0]
    NCORES = 8
    NTOK = B * S
    T = NTOK // NCORES
    ff_sh = FF // NCORES
    dm_sh = DM // NCORES

    xf = np.ascontiguousarray(x.reshape(NTOK, DM))
    wgT = np.ascontiguousarray(w_gate.T)
    wuT = np.ascontiguousarray(w_up.T)
    wdT = np.ascontiguousarray(w_down.T)
    nc = _get_program(T, DM, FF, NCORES, ff_sh, dm_sh)

    in_maps = []
    for c in range(NCORES):
        xs = np.ascontiguousarray(xf[c * T : (c + 1) * T])
        in_maps.append(
            {
                "x": xs,
                "xt": np.ascontiguousarray(xs.T),
                "wgt": wgT,
                "wut": wuT,
                "wdt": wdT,
                "wg_sh": np.ascontiguousarray(w_gate[c * ff_sh : (c + 1) * ff_sh]),
                "wu_sh": np.ascontiguousarray(w_up[c * ff_sh : (c + 1) * ff_sh]),
                "wd_sh": np.ascontiguousarray(w_down[c * dm_sh : (c + 1) * dm_sh]),
            }
        )

    res = run_bass_kernel_spmd(
        nc, in_maps, core_ids=list(range(NCORES)), trace=TRACE
    )
    global LAST_RESULTS
    LAST_RESULTS = res
    out = np.empty((NTOK, DM), dtype=np.float32)
    for c in range(NCORES):
        out[c * T : (c + 1) * T] = res.results[c]["out_t"].T
    return out.reshape(B, S, DM)
